# revision 13
# baseline (speedup 1.0000x reference)
"""Trainium2 Bass kernel for BinOverlapPredictionFromMaxProj (segment max + masked mean).

Full computation:
  ptm: (32, 8, 30, 1, 72, 72) f32, mem_mask: (32, 8, 30) bool
  n = 32*8 = 256 rows; per row: max over 5184-feature axis per mem (30), then
  masked mean over mems -> out (256,) f32.

Sharding: data-parallel over the 256 fused rows across 8 cores (32 rows each).
Per core: 960 segments x 5184 features (~19.9 MB) -> memory-bound; the DMA
stream (19.9 MB at ~425 GB/s ~= 47 us) is the roofline.

Device plan per core (v5, pair-aligned layout):
  The host pre-arranges each core's 1920 half-segments (960 segs x 2 halves of
  2592 floats) into a (128, 15, 2592) buffer so that
    - cols 0..13 of partition p hold 7 WHOLE segments (halves adjacent), and
    - col 14 holds one stray HALF; row r owns partitions 4r..4r+3, its two
      stray segments' halves sitting at col 14 of those four partitions.
  This makes the segment pair-max a within-partition stride-2 tensor_max and
  the cross-partition masked row-sum a single PE matmul (128->32 block-sum
  weights), eliminating the SBUF repartition DMA of the earlier design.

  Loads: col 14 first (w=1), then 7 loads of col pairs (w=2), all on the
  gpsimd SWDGE queue (HWDGE/sync measured ~2x slower for big loads; f32
  keeps write packets >= 10 KB, below which SDMA engine 15 falls behind).
  Vector reduces each load as it lands (w=2 3D-AP reduces measured at
  2751 ns/col vs 3394 ns for flat w=1 mid-stream).

  Stray path (off critical path): statS (128,1) -> PE transpose -> (1,128)
  PSUM -> stride-2 max + mask + pair-sum on one partition -> straysumT (1,32).
  Main tail: stride-2 pairmax -> (128,7), mask-mul, row-partial reduce,
  PE matmul partial.T @ W1 -> psum (1,32), add straysum, mul 1/count,
  single-descriptor 128B out DMA.

The walrus codegen allows only ONE attached sync wait per instruction, so
TileContext's kernel-tail Drain is rewritten (waits beyond the first become
standalone wait_ge) and PE "warmup" matmuls touch each constant tile so later
matmuls carry a single data wait. See _patch_tile_drain().
"""

import sys

import numpy as np

if "/opt/trn_rl_repo" not in sys.path:
    sys.path.insert(0, "/opt/trn_rl_repo")

NCORES = 8
NF, NS, NMEM, FEAT = 32, 8, 30, 5184
N = NF * NS  # 256
ROWS = N // NCORES  # 32 rows per core
SEGS = ROWS * NMEM  # 960 segments per core
PPART = 128  # partitions
HALF = FEAT // 2  # 2592 floats per half-segment
HPP = SEGS * 2 // PPART  # 15 half-segments per partition
NWHOLE = 7  # whole segments per partition (cols 0..13)

_NC_CACHE = {}


def _patch_tile_drain():
    """Split the kernel-tail Drain's semaphore waits into standalone wait_ge
    instructions (one wait per instruction), to fit the walrus per-instruction
    sync-wait limit."""
    import concourse.tile as tile
    from concourse.vector_clock import ScopedClock

    if getattr(tile.TileContext._drain_and_barrier, "_single_wait_patch", False):
        return

    def _drain_and_barrier(self, tick_clock, wait_clock):
        drain_inst = self.nc.sync.drain()
        wait_clock.add_sem_waits(
            drain_inst.ins, ScopedClock({None: tick_clock.global_clock})
        )
        si = drain_inst.ins.sync_info
        waits = list(si.on_wait) if si is not None else []
        if len(waits) > 1:
            si.on_wait = [waits[0]]
            by_name = {h.name: h for h in self.sems.allocated().values()}
            for w in waits[1:]:
                self.nc.sync.wait_ge(by_name[w.ant_name], w.wait_value)

        self.nc.all_engine_barrier()
        assert self.sems is not None
        popped = self.nc._tile_sem_poison_stack.pop()
        assert popped is self._sem_poison
        self.nc.clear_and_free_semaphores(list(self.sems.allocated().values()))
        self.nc.all_engine_barrier()

    _drain_and_barrier._single_wait_patch = True
    tile.TileContext._drain_and_barrier = _drain_and_barrier


def _build_nc():
    import concourse.bass as bass
    import concourse.tile as tile
    from concourse import mybir
    from concourse.bass import MemorySpace

    _patch_tile_drain()

    f32 = mybir.dt.float32
    X = mybir.AxisListType.X

    nc = bass.Bass("TRN2")
    ptm = nc.dram_tensor("ptm", [PPART, HPP, HALF], f32, kind="ExternalInput")
    maskA = nc.dram_tensor("maskA", [PPART, NWHOLE], f32, kind="ExternalInput")
    maskS2 = nc.dram_tensor("maskS2", [1, 2 * ROWS], f32, kind="ExternalInput")
    maskT1 = nc.dram_tensor("maskT1", [1, ROWS * NMEM], f32, kind="ExternalInput")
    w1 = nc.dram_tensor("w1", [PPART, ROWS], f32, kind="ExternalInput")
    ident = nc.dram_tensor("ident", [PPART, PPART], f32, kind="ExternalInput")
    out = nc.dram_tensor("out", [1, ROWS], f32, kind="ExternalOutput")

    with tile.TileContext(nc) as tc:
        with (
            tc.tile_pool(name="data", bufs=1) as dpool,
            tc.tile_pool(name="small", bufs=1) as spool,
            tc.tile_pool(name="psum", bufs=1, space=MemorySpace.PSUM) as ppool,
        ):
            # Constants / small inputs, all early on the scalar HWDGE queue.
            w1_t = spool.tile([PPART, ROWS], f32)
            nc.scalar.dma_start(out=w1_t[:], in_=w1[:])
            ident_t = spool.tile([PPART, PPART], f32)
            nc.scalar.dma_start(out=ident_t[:], in_=ident[:])
            maskA_t = spool.tile([PPART, NWHOLE], f32)
            nc.scalar.dma_start(out=maskA_t[:], in_=maskA[:])
            maskS2_t = spool.tile([1, 2 * ROWS], f32)
            nc.scalar.dma_start(out=maskS2_t[:], in_=maskS2[:])
            maskT1_t = spool.tile([1, ROWS * NMEM], f32)
            nc.scalar.dma_start(out=maskT1_t[:], in_=maskT1[:])

            # PE warmups: touch each constant tile once so later PE ops carry
            # a single data wait (walrus one-wait-per-instruction limit).
            warm = ppool.tile([1, ROWS], f32)
            nc.tensor.matmul(warm[:], ident_t[:, 0:1], ident_t[:, 0:ROWS],
                             start=True, stop=True)
            nc.tensor.matmul(warm[:], w1_t[:, 0:1], w1_t[:],
                             start=True, stop=True)

            # Stray column load first; its processing overlaps the stream.
            dS = dpool.tile([PPART, 1, HALF], f32, name="dataS", tag="dataS")
            nc.gpsimd.dma_start(out=dS[:], in_=ptm[:, HPP - 1 : HPP, :])
            statS = spool.tile([PPART, 1], f32)
            nc.vector.reduce_max(out=statS[:], in_=dS[:], axis=X)

            # Row counts in transposed (1, 32) layout, off critical path.
            m3 = maskT1_t[:].rearrange("one (r m) -> one r m", m=NMEM)
            cntT = spool.tile([1, ROWS], f32)
            nc.vector.reduce_sum(out=cntT[:], in_=m3, axis=X)
            rcntT = spool.tile([1, ROWS], f32)
            nc.vector.reciprocal(out=rcntT[:], in_=cntT[:])
            # DVE warm-touches of the const tiles: later DVE ops then carry
            # at most one (PE/PSUM) wait - walrus allows only one per
            # instruction.
            touch = spool.tile([1, 2], f32)
            nc.vector.tensor_copy(out=touch[0:1, 0:1], in_=maskA_t[0:1, 0:1])
            nc.vector.tensor_copy(out=touch[0:1, 1:2], in_=maskS2_t[0:1, 0:1])

            # PE transpose of the stray half-maxes to one partition.
            strayP = ppool.tile([1, PPART], f32)
            nc.tensor.transpose(strayP[:], statS[:], ident_t[:])

            # Main loads: 7 col-pairs, each reduced as one w=2 3D-AP reduce.
            stats13 = spool.tile([PPART, 2 * NWHOLE], f32)
            straysum = spool.tile([1, ROWS], f32)
            for k in range(NWHOLE):
                d = dpool.tile(
                    [PPART, 2, HALF], f32, name="data2", tag="data2", bufs=NWHOLE
                )
                nc.gpsimd.dma_start(out=d[:], in_=ptm[:, 2 * k : 2 * k + 2, :])
                nc.vector.reduce_max(
                    out=stats13[:, 2 * k : 2 * k + 2], in_=d[:], axis=X
                )
                if k == 2:
                    # Stray path on DVE, inserted mid-stream: copy the PSUM
                    # transpose to SBUF (TT may read only one PSUM input),
                    # then pairwise max, mask, pair-sum -> straysum (1,32).
                    strayC = spool.tile([1, PPART], f32)
                    nc.vector.tensor_copy(out=strayC[:], in_=strayP[:])
                    strayM = spool.tile([1, 2 * ROWS], f32)
                    nc.vector.tensor_max(
                        out=strayM[:],
                        in0=strayC[0:1, 0 : PPART : 2],
                        in1=strayC[0:1, 1 : PPART : 2],
                    )
                    strayMM = spool.tile([1, 2 * ROWS], f32)
                    nc.vector.tensor_mul(
                        out=strayMM[:], in0=strayM[:], in1=maskS2_t[:]
                    )
                    sv = strayMM[:].rearrange("one (r two) -> one r two", two=2)
                    nc.vector.reduce_sum(out=straysum[:], in_=sv, axis=X)

            # Tail: pairmax -> mask -> row partials -> PE block-sum matmul.
            segmax = spool.tile([PPART, NWHOLE], f32)
            nc.vector.tensor_max(
                out=segmax[:],
                in0=stats13[:, 0 : 2 * NWHOLE : 2],
                in1=stats13[:, 1 : 2 * NWHOLE : 2],
            )
            masked = spool.tile([PPART, NWHOLE], f32)
            nc.vector.tensor_mul(out=masked[:], in0=segmax[:], in1=maskA_t[:])
            partial = spool.tile([PPART, 1], f32)
            nc.vector.reduce_sum(out=partial[:], in_=masked[:], axis=X)

            acc = ppool.tile([1, ROWS], f32)
            nc.tensor.matmul(acc[:], partial[:], w1_t[:], start=True, stop=True)

            tmp = spool.tile([1, ROWS], f32)
            nc.vector.tensor_add(out=tmp[:], in0=acc[:], in1=straysum[:])
            res = spool.tile([1, ROWS], f32)
            nc.vector.tensor_mul(out=res[:], in0=tmp[:], in1=rcntT[:])
            nc.scalar.dma_start(out=out[:], in_=res[:])

    return nc


def _get_nc():
    if "nc" not in _NC_CACHE:
        _NC_CACHE["nc"] = _build_nc()
    return _NC_CACHE["nc"]


def _host_layout():
    """Pair-aligned half-segment permutation and mask/weight constants.

    idx[p, j] = half-segment index (seg*2 + half, within one core's 1920)
    placed at (partition p, col j). Row r owns partitions 4r..4r+3; each
    holds 7 whole segments (cols 0..13, halves adjacent) plus one stray
    half at col 14 (segs 28/29 of the row, halves on partition pairs).
    """
    idx = np.empty((PPART, HPP), dtype=np.int64)
    w1row = np.zeros((PPART, ROWS), dtype=np.float32)
    for r in range(ROWS):
        for j in range(4):
            p = 4 * r + j
            w1row[p, r] = 1.0
            for k in range(NWHOLE):
                seg = r * NMEM + 7 * j + k
                idx[p, 2 * k] = 2 * seg
                idx[p, 2 * k + 1] = 2 * seg + 1
        idx[4 * r + 0, 14] = 2 * (r * NMEM + 28)
        idx[4 * r + 1, 14] = 2 * (r * NMEM + 28) + 1
        idx[4 * r + 2, 14] = 2 * (r * NMEM + 29)
        idx[4 * r + 3, 14] = 2 * (r * NMEM + 29) + 1
    ident = np.eye(PPART, dtype=np.float32)
    return idx.reshape(-1), w1row, ident


_IDX, _W1ROW, _IDENT = _host_layout()


def make_in_maps(ptm, mem_mask):
    ptm = np.ascontiguousarray(np.asarray(ptm, dtype=np.float32))
    mask = np.asarray(mem_mask).reshape(N, NMEM).astype(np.float32)
    halves = ptm.reshape(N * NMEM * 2, HALF)

    in_maps = []
    for i in range(NCORES):
        core_halves = halves[i * SEGS * 2 : (i + 1) * SEGS * 2]
        shard = core_halves[_IDX].reshape(PPART, HPP, HALF)
        m = mask[i * ROWS : (i + 1) * ROWS]  # (32, 30)
        maskA = np.empty((PPART, NWHOLE), dtype=np.float32)
        for j in range(4):
            maskA[j::4] = m[:, 7 * j : 7 * j + 7]
        maskS2 = np.ascontiguousarray(m[:, 28:30].reshape(1, 2 * ROWS))
        maskT1 = np.ascontiguousarray(m.reshape(1, ROWS * NMEM))
        in_maps.append(
            {
                "ptm": shard,
                "maskA": maskA,
                "maskS2": maskS2,
                "maskT1": maskT1,
                "w1": _W1ROW,
                "ident": _IDENT,
            }
        )
    return in_maps


def _ensure_ntff_hook():
    """Register the axon NTFF profiling hook (the container's antenv lacks
    axon_hooks; synthesize it from trn_agent_boot), and stub the artifact
    upload which has no bucket access here."""
    import types

    try:
        from antenv.axon_hooks import get_axon_ntff_profile_hook  # noqa: F401
    except ImportError:
        import antenv
        from trn_agent_boot.trn_boot import _ntff_profile_via_ctypes

        mod = types.ModuleType("antenv.axon_hooks")
        mod._hook = _ntff_profile_via_ctypes("/opt/axon/libaxon_pjrt.so")
        mod.set_axon_ntff_profile_hook = lambda h: setattr(mod, "_hook", h)
        mod.get_axon_ntff_profile_hook = lambda: mod._hook
        sys.modules["antenv.axon_hooks"] = mod
        antenv.axon_hooks = mod

    from concourse import bass_utils

    if not getattr(bass_utils.upload_artifacts, "_stubbed", False):
        def _no_upload(tmpdir):
            return str(tmpdir)

        _no_upload._stubbed = True
        bass_utils.upload_artifacts = _no_upload


def run(ptm, mem_mask, trace=False):
    from concourse.bass_utils import run_bass_kernel_spmd

    if trace:
        _ensure_ntff_hook()

    in_maps = make_in_maps(ptm, mem_mask)

    nc = _get_nc()
    kr = run_bass_kernel_spmd(nc, in_maps, list(range(NCORES)), trace=trace)
    out = np.concatenate(
        [np.asarray(kr.results[i]["out"]).reshape(ROWS) for i in range(NCORES)]
    )
    return out.astype(np.float32), kr


def kernel(ptm, mem_mask):
    out, _ = run(ptm, mem_mask, trace=False)
    return out
